# revision 21
# baseline (speedup 1.0000x reference)
"""AdaptiveSpectralDNA Trainium2 kernel: 8-core data-parallel SIREN MLP.

Feature-major activations (features on partitions, rows on free dim) so no
transposes are needed.  Hidden/final matmuls in fp16 (same 10-bit mantissa
as tf32/float32r, but 2-byte weights load fast); layer-0 and its large
+-300 rad arguments in fp32; the omega net in float32r.  Biases enter PSUM
exactly via K=2 fp16 hi/lo rank-1 matmuls against a constant ones vector.
sin(omega*(Wx+b)) = sin2pi(frac(t2)), t2 = (omega/2pi)*(Wx+b);
frac via the fp32 magic-number round, sin2pi via a BIR patch (bass has no
enum for it; it shares an ACT table set with sigmoid so no table switches).
"""
import os
import numpy as np

N = 524288
NHL = 4
NC = 8
NCORE = N // NC          # 65536 rows per core
R = 512                  # rows per tile (one PSUM bank of fp32)
T = NCORE // R           # 128 tiles
MAGIC = float(1.5 * 2**23)
INV2PI = float(1.0 / (2.0 * np.pi))
MIN_O, MAX_O = 10.0, 100.0

_CACHE = {}


def _build():
    import concourse.bass as bass
    import concourse.mybir as mybir
    from concourse.tile import TileContext

    F32 = mybir.dt.float32
    F32R = mybir.dt.float32r
    F16 = mybir.dt.float16
    A = mybir.ActivationFunctionType
    ALU = mybir.AluOpType

    nc = bass.Bass()
    coordsT = nc.declare_dram_parameter("coordsT", [4, NCORE], F32R, isOutput=False)
    coordsTF = nc.declare_dram_parameter("coordsTF", [4, NCORE], F32, isOutput=False)
    WH_e = nc.declare_dram_parameter("WH", [128, 16 * 128], F16, isOutput=False)
    W0_e = nc.declare_dram_parameter("W0", [4, 256], F32, isOutput=False)
    OW1_e = nc.declare_dram_parameter("OW1", [4, 64], F32R, isOutput=False)
    OW2R_e = nc.declare_dram_parameter("OW2R", [64, 128], F32R, isOutput=False)
    WF_e = nc.declare_dram_parameter("WF", [128, 2], F16, isOutput=False)
    BH2_e = nc.declare_dram_parameter("BH2", [2, 10 * 128], F16, isOutput=False)
    ONES2_e = nc.declare_dram_parameter("ONES2", [2, R], F16, isOutput=False)
    OB1_e = nc.declare_dram_parameter("OB1", [64, 1], F32, isOutput=False)
    OB2C_e = nc.declare_dram_parameter("OB2C", [128, 1], F32, isOutput=False)
    MAGC_e = nc.declare_dram_parameter("MAGC", [128, 1], F32, isOutput=False)
    BCOL_e = nc.declare_dram_parameter("BCOL", [128, 10], F32, isOutput=False)
    out_e = nc.declare_dram_parameter("out", [NCORE], F32, isOutput=True)

    with TileContext(nc) as tc:
        with (
            tc.tile_pool(name="wpool", bufs=1) as wpool,
            tc.tile_pool(name="cpool", bufs=8) as cpool,
            tc.tile_pool(name="work", bufs=8) as work,
            tc.tile_pool(name="uvt", bufs=5) as uvt,
            tc.tile_pool(name="ompool", bufs=10) as ompool,
            tc.tile_pool(name="ypool", bufs=10) as ypool,
            tc.tile_pool(name="opool", bufs=3) as opool,
            tc.tile_pool(name="pz", bufs=8, space="PSUM") as pz,
        ):
            WH = wpool.tile([128, 16 * 128], F16)
            W0 = wpool.tile([4, 256], F32)
            OW1 = wpool.tile([4, 64], F32R)
            OW2R = wpool.tile([64, 128], F32R)
            WF = wpool.tile([128, 2], F16)
            BH2 = wpool.tile([2, 10 * 128], F16)
            ONES2 = wpool.tile([2, R], F16)
            OB1 = wpool.tile([64, 1], F32)
            OB2C = wpool.tile([128, 1], F32)
            MAGC = wpool.tile([128, 1], F32)
            BCOL = wpool.tile([128, 10], F32)
            MAGBC = wpool.tile([128, 2 * R], F32)
            nc.gpsimd.memset(MAGBC[:], MAGIC)
            for dst, src in [(WH, WH_e), (W0, W0_e), (OW1, OW1_e),
                             (OW2R, OW2R_e), (WF, WF_e), (BH2, BH2_e),
                             (ONES2, ONES2_e), (OB1, OB1_e), (OB2C, OB2C_e),
                             (MAGC, MAGC_e), (BCOL, BCOL_e)]:
                nc.sync.dma_start(out=dst[:], in_=src[:])

            def whb(i, kh, fh):
                j = (i * 2 + kh) * 2 + fh
                return WH[:, j * 128:(j + 1) * 128]

            def emit_omega(st):
                t = st["t"]
                c = cpool.tile([4, R], F32R, name="c")
                nc.sync.dma_start(out=c[:], in_=coordsT[:, t * R:(t + 1) * R])
                cf = cpool.tile([4, R], F32, name="cf")
                nc.sync.dma_start(out=cf[:], in_=coordsTF[:, t * R:(t + 1) * R])
                st["cf"] = cf
                phx = pz.tile([128, R], F32, name="pzz")
                nc.tensor.matmul(phx[0:64, :], OW1[:], c[:], start=True, stop=True)
                h = work.tile([64, R], F32R, name="h")
                nc.scalar.activation(h[:], phx[0:64, :], A.Relu, bias=OB1[:, 0:1])
                pux = pz.tile([128, R], F32, name="pzz")
                nc.tensor.matmul(pux[:], OW2R[:], h[:], start=True, stop=True)
                sg = work.tile([128, R], F32, name="sg")
                nc.scalar.activation(sg[:], pux[:], A.Sigmoid,
                                     bias=OB2C[:, 0:1])
                om2 = ompool.tile([128, R], F32, name="om2")
                nc.vector.tensor_scalar(om2[:], sg[:], (MAX_O - MIN_O) * INV2PI,
                                        MIN_O * INV2PI, ALU.mult, ALU.add)
                st["om2"] = om2

            def emit_stage_mms(sts, s):
                for st in sts:
                    st["pz0"] = pz.tile([128, R], F32, name="pzz")
                    st["pz1"] = pz.tile([128, R], F32, name="pzz")
                    for fh, zs in ((0, st["pz0"][:]), (1, st["pz1"][:])):
                        if s == 0:
                            nc.tensor.matmul(zs, W0[:, fh * 128:(fh + 1) * 128],
                                             st["cf"][:], start=True, stop=True)
                        else:
                            i = s - 1
                            nc.tensor.matmul(zs, whb(i, 0, fh), st["y"][:, 0:R],
                                             start=True, stop=False)
                            nc.tensor.matmul(zs, whb(i, 1, fh), st["y"][:, R:2 * R],
                                             start=False, stop=True)

            def emit_stage_elem(st, s):
                om2 = st["om2"]
                tt = uvt.tile([128, 2 * R], F32, name="tt")
                for fh, zs in ((0, st["pz0"][:]), (1, st["pz1"][:])):
                    nc.vector.scalar_tensor_tensor(
                        tt[:, fh * R:(fh + 1) * R], zs,
                        BCOL[:, 2 * s + fh:2 * s + fh + 1],
                        om2[:], ALU.add, ALU.mult)
                u = uvt.tile([128, 2 * R], F32, name="u")
                if s in (0, 1, 2):
                    nc.scalar.activation(u[:], tt[:], A.Identity, bias=MAGC[:, 0:1])
                else:
                    nc.gpsimd.tensor_tensor(out=u[:], in0=tt[:], in1=MAGBC[:],
                                            op=ALU.add)
                v = uvt.tile([128, 2 * R], F32, name="v")
                if s in (2, 3):
                    k = uvt.tile([128, 2 * R], F32, name="k")
                    nc.gpsimd.tensor_tensor(out=k[:], in0=u[:], in1=MAGBC[:],
                                            op=ALU.subtract)
                    nc.gpsimd.tensor_tensor(out=v[:], in0=k[:], in1=tt[:],
                                            op=ALU.subtract)
                else:
                    nc.vector.scalar_tensor_tensor(v[:], u[:], MAGIC, tt[:],
                                                   ALU.subtract, ALU.subtract)
                y2 = ypool.tile([128, 2 * R], F16, name="y")
                nc.scalar.activation(y2[:], v[:], A.Arctan, scale=-1.0)
                st["y"] = y2

            def emit_final(st):
                pof = pz.tile([128, R], F32, name="pzz")
                poo = pof[0:1, :]
                y = st["y"]
                nc.tensor.matmul(poo, WF[:, 0:1], y[:, 0:R], start=True, stop=False)
                nc.tensor.matmul(poo, WF[:, 1:2], y[:, R:2 * R], start=False,
                                 stop=True)
                ot = opool.tile([1, R], F32, name="ot")
                nc.scalar.activation(ot[:], poo, A.Copy)
                t = st["t"]
                nc.sync.dma_start(out=out_e[t * R:(t + 1) * R], in_=ot[0:1, :])

            GW = int(os.environ.get("KB_GW", "6"))
            assert T % GW == 0, (T, GW)
            for tq in range(T // GW):
                sts = [{"t": GW * tq + k} for k in range(GW)]
                for st in sts:
                    emit_omega(st)
                for s in range(5):
                    emit_stage_mms(sts, s)
                    for st in sts:
                        emit_stage_elem(st, s)
                for st in sts:
                    emit_final(st)

    _split_multiwaits(nc, mybir)
    return nc


def _split_multiwaits(nc, mybir):
    """This walrus build accepts only ONE sync wait per instruction: splice
    extra waits onto single-wait same-engine NOPs placed just before the
    over-subscribed instruction (engine streams are in-order)."""
    ctr = 0
    for fn in nc.m.functions:
        for bb in fn.blocks:
            insts = list(bb.instructions)
            out = []
            changed = False
            for inst in insts:
                si = inst.sync_info
                waits = list(si.on_wait) if si and si.on_wait else []
                if len(waits) > 1:
                    changed = True
                    for w in waits[:-1]:
                        ctr += 1
                        nop = mybir.InstNoOp(
                            name=f"I-waitfix-{ctr}",
                            engine=inst.engine,
                            sync_info=mybir.SyncInfo(on_wait=[w], on_update=[]),
                        )
                        out.append(nop)
                    inst.sync_info = mybir.SyncInfo(
                        on_wait=[waits[-1]], on_update=list(si.on_update)
                    )
                out.append(inst)
            if changed:
                bb.instructions = out
    return nc


def _install_sin2pi_patch():
    import concourse.bass2jax as b2j
    import concourse.bass_utils as bu
    from concourse.bass_utils import compile_bir_kernel

    def patched(bir_json, tmpdir, neff_name="file.neff"):
        bir_json = bir_json.replace(b'"func":"Arctan"', b'"func":"Sin2pi"')
        return compile_bir_kernel(bir_json, tmpdir, neff_name)

    b2j.compile_bir_kernel = patched
    if os.environ.get("KB_LDWOPT"):
        orig_run = bu.run_command

        def run_patched(argv, **kwargs):
            argv = ["--enable-ldw-opt=true" if a == "--enable-ldw-opt=false" else a
                    for a in argv]
            return orig_run(argv, **kwargs)

        if getattr(bu.run_command, "__name__", "") != "run_patched":
            bu.run_command = run_patched


def _hi_lo_f16(x):
    hi = x.astype(np.float16)
    lo = ((x - hi.astype(np.float32)) * 256.0).astype(np.float16)
    return hi, lo


def _prep_inputs(coords, ow1, ob1, ow2, ob2, w0, b0, wh, bh, wf, bf):
    coords = np.asarray(coords, np.float32)
    wh = np.asarray(wh, np.float32)
    WH = np.empty((128, 16 * 128), np.float16)
    for i in range(NHL):
        for kh in range(2):
            for fh in range(2):
                j = (i * 2 + kh) * 2 + fh
                WH[:, j * 128:(j + 1) * 128] = wh[i, kh * 128:(kh + 1) * 128,
                                                  fh * 128:(fh + 1) * 128].astype(np.float16)
    W0 = np.asarray(w0, np.float32)
    OW1 = np.asarray(ow1, np.float32)
    OW2R = np.tile(np.asarray(ow2, np.float32), (1, 128))
    wf = np.asarray(wf, np.float32)
    WF = np.stack([wf[0:128, 0], wf[128:256, 0]], axis=1).astype(np.float16)
    # biases: K=2 hi/lo fp16 rank-1 rows; lo scaled by 256 (fp16 normal range)
    ball = np.empty((10, 128), np.float32)
    b0 = np.asarray(b0, np.float32)
    bh = np.asarray(bh, np.float32)
    ball[0] = b0[0:128]
    ball[1] = b0[128:256]
    for i in range(NHL):
        for fh in range(2):
            ball[2 + 2 * i + fh] = bh[i, fh * 128:(fh + 1) * 128]
    BH2 = np.empty((2, 10 * 128), np.float16)
    for j in range(10):
        hi, lo = _hi_lo_f16(ball[j])
        BH2[0, j * 128:(j + 1) * 128] = hi
        BH2[1, j * 128:(j + 1) * 128] = lo
    ONES2 = np.empty((2, R), np.float16)
    ONES2[0] = 1.0
    ONES2[1] = 1.0 / 256.0
    OB1 = np.asarray(ob1, np.float32).reshape(64, 1)
    OB2C = np.full((128, 1), np.float32(np.asarray(ob2).reshape(-1)[0]), np.float32)
    shared = {"WH": WH, "W0": W0, "OW1": OW1, "OW2R": OW2R, "WF": WF,
              "BH2": BH2, "ONES2": ONES2, "OB1": OB1, "OB2C": OB2C,
              "MAGC": np.full((128, 1), MAGIC, np.float32),
              "BCOL": np.ascontiguousarray(ball.T)}
    in_maps = []
    for cix in range(NC):
        shard = coords[cix * NCORE:(cix + 1) * NCORE]
        m = dict(shared)
        m["coordsT"] = np.ascontiguousarray(shard.T)
        m["coordsTF"] = m["coordsT"]
        in_maps.append(m)
    return in_maps, np.float32(np.asarray(bf).reshape(-1)[0])


def kernel(coords, ow1, ob1, ow2, ob2, w0, b0, wh, bh, wf, bf, _trace=False):
    from concourse.bass_utils import run_bass_kernel_spmd

    _install_sin2pi_patch()
    if "nc" not in _CACHE:
        _CACHE["nc"] = _build()
    nc = _CACHE["nc"]
    in_maps, bf_v = _prep_inputs(coords, ow1, ob1, ow2, ob2, w0, b0,
                                 wh, bh, wf, bf)
    res = run_bass_kernel_spmd(nc, in_maps, core_ids=list(range(NC)),
                               trace=_trace)
    _CACHE["last_res"] = res
    outs = [np.asarray(res.results[i]["out"]).reshape(NCORE) for i in range(NC)]
    full = np.concatenate(outs) + bf_v
    return full.reshape(N, 1).astype(np.float32)
